# revision 1
# baseline (speedup 1.0000x reference)
"""Multi-head attention (B=2, S=2048, D=1024, H=16) on 8 trn2 NeuronCores.

Sharding: head-parallel. Core c owns heads {2c, 2c+1} (= feature rows
[128c, 128c+128) of the QKV projections / columns of Wo). Each core:
  - projects full q/k/v (pre-transposed + bf16-cast on host) against its
    128-column slice of Wq/Wk/Wv,
  - runs softmax(QK^T * s) @ V for its 4 (batch, head) pairs using a
    transposed-score layout (keys on partitions) so no on-chip transposes
    are needed,
  - computes its additive partial of the output projection
    (attn_heads @ Wo[:, cols].T) in row-parallel fashion.
Host sums the 8 partials and adds the (bo + bv @ Wo.T) constant, which is
where the bv bias lands after the softmax-normalization algebra.
"""

import sys

for _p in ("/opt/trn_rl_repo",):
    if _p not in sys.path:
        try:
            import concourse  # noqa: F401
            break
        except ImportError:
            sys.path.insert(0, _p)

import numpy as np
import ml_dtypes

import concourse.bass as bass
import concourse.tile as tile
from concourse import mybir
from concourse.bass_utils import run_bass_kernel_spmd

BF16 = mybir.dt.bfloat16
F32 = mybir.dt.float32
AF = mybir.ActivationFunctionType

B, S, D, H, DH = 2, 2048, 1024, 16, 64
NCORES = 8
T = B * S              # 4096 tokens
HC = 128               # head-columns per core (2 heads x 64)
KO = D // 128          # 8 contraction tiles for projections
SCALE = DH ** -0.5     # 0.125

_NC = None


def _split_multiwaits(nc, maxw=1):
    """Walrus codegen in this container rejects Drain instructions carrying
    more than ~2 semaphore waits ("Too many sync wait commands"). Move the
    excess waits onto preceding NoOps on the same engine."""
    ctr = 0
    for f in nc.m.functions:
        for bb in f.blocks:
            newlist = []
            changed = False
            for inst in bb.instructions:
                si = inst.sync_info
                if (si is not None and si.on_wait and len(si.on_wait) > maxw):
                    waits = list(si.on_wait)
                    for j in range(maxw, len(waits), maxw):
                        nop = mybir.InstNoOp(name=f"splitw-{ctr}", ins=[], outs=[])
                        ctr += 1
                        nop.engine = inst.engine
                        nop.sync_info = mybir.SyncInfo(
                            on_wait=list(waits[j:j + maxw]), on_update=[])
                        newlist.append(nop)
                    inst.sync_info = mybir.SyncInfo(
                        on_wait=waits[:maxw], on_update=list(si.on_update))
                    changed = True
                newlist.append(inst)
            if changed:
                bb.instructions = newlist
    return ctr


def _build(split=True):
    nc = bass.Bass()

    qT = nc.declare_dram_parameter("qT", [D, T], BF16, isOutput=False)
    kT = nc.declare_dram_parameter("kT", [D, T], BF16, isOutput=False)
    vT = nc.declare_dram_parameter("vT", [D, T], BF16, isOutput=False)
    wq = nc.declare_dram_parameter("wq", [D, HC], BF16, isOutput=False)
    wk = nc.declare_dram_parameter("wk", [D, HC], BF16, isOutput=False)
    wv = nc.declare_dram_parameter("wv", [D, HC], BF16, isOutput=False)
    bq = nc.declare_dram_parameter("bq", [HC, 1], F32, isOutput=False)
    bk = nc.declare_dram_parameter("bk", [HC, 1], F32, isOutput=False)
    wo = nc.declare_dram_parameter("wo", [HC, D], BF16, isOutput=False)
    out = nc.declare_dram_parameter("out", [T, D], F32, isOutput=True)

    qT3 = qT.rearrange("(ko p) n -> p ko n", p=128)
    kT3 = kT.rearrange("(ko p) n -> p ko n", p=128)
    vT3 = vT.rearrange("(ko p) n -> p ko n", p=128)
    wq3 = wq.rearrange("(ko p) m -> p ko m", p=128)
    wk3 = wk.rearrange("(ko p) m -> p ko m", p=128)
    wv3 = wv.rearrange("(ko p) m -> p ko m", p=128)

    with tile.TileContext(nc) as tc:
        with (
            tc.tile_pool(name="consts", bufs=1) as consts,
            tc.tile_pool(name="big", bufs=1) as big,
            tc.tile_pool(name="small", bufs=2) as small,
        ):
            # ---- persistent SBUF state ----
            wq_s = consts.tile([128, KO, 128], BF16, tag="wq")
            wk_s = consts.tile([128, KO, 128], BF16, tag="wk")
            wv_s = consts.tile([128, KO, 128], BF16, tag="wv")
            wo_s = consts.tile([HC, D], BF16, tag="wo")
            bq_s = consts.tile([HC, 1], F32, tag="bq")
            bk_s = consts.tile([HC, 1], F32, tag="bk")
            nc.sync.dma_start(wq_s[:], wq3[:])
            nc.sync.dma_start(wk_s[:], wk3[:])
            nc.sync.dma_start(wv_s[:], wv3[:])
            nc.sync.dma_start(wo_s[:], wo[:])
            nc.sync.dma_start(bq_s[:], bq[:])
            nc.sync.dma_start(bk_s[:], bk[:])

            # Per-head Q/K buffers zero-padded to 128 partitions: a K=64
            # matmul runs at half the K=128 streaming rate on this silicon,
            # so scores contract over 128 rows with rows 64-127 always zero.
            QTp = [big.tile([128, T], BF16, tag=f"QTp{h}", name=f"QTp{h}")
                   for h in range(2)]
            KTp = [big.tile([128, T], BF16, tag=f"KTp{h}", name=f"KTp{h}")
                   for h in range(2)]
            for h in range(2):
                nc.vector.memset(QTp[h][64:128, :], 0.0)
                nc.vector.memset(KTp[h][64:128, :], 0.0)
            attnT = big.tile([HC, T], BF16, tag="attnT")
            # [V | 1] per (batch, local head): k-tokens on partitions,
            # 16 k-tiles x (64 dh + ones) along free.
            V1 = [[big.tile([128, 16 * 65], BF16, tag=f"V1_{b}_{h}",
                            name=f"V1_{b}_{h}")
                   for h in range(2)] for b in range(B)]
            for b in range(B):
                for h in range(2):
                    ones_col = V1[b][h].rearrange("p (t s) -> p t s", s=65)[:, :, 64]
                    nc.vector.memset(ones_col, 1.0)

            # ---- phase A: projections ----
            with (
                tc.tile_pool(name="instage", bufs=3) as instage,
                tc.tile_pool(name="pp_qk", bufs=2, space="PSUM") as pp_qk,
                tc.tile_pool(name="pp_v", bufs=2, space="PSUM") as pp_v,
            ):
                for c in range(T // 512):
                    cs = bass.ts(c, 512)
                    q_in = instage.tile([128, KO, 512], BF16, tag="q_in")
                    k_in = instage.tile([128, KO, 512], BF16, tag="k_in")
                    v_in = instage.tile([128, KO, 512], BF16, tag="v_in")
                    nc.sync.dma_start(q_in[:], qT3[:, :, cs])
                    nc.sync.dma_start(k_in[:], kT3[:, :, cs])
                    nc.sync.dma_start(v_in[:], vT3[:, :, cs])

                    ps_q = pp_qk.tile([HC, 512], F32, tag="ps_qk")
                    for ko in range(KO):
                        nc.tensor.matmul(ps_q[:], wq_s[:, ko, :], q_in[:, ko, :],
                                         start=(ko == 0), stop=(ko == KO - 1))
                    nc.vector.tensor_scalar_add(QTp[0][0:64, cs], ps_q[0:64, :],
                                                bq_s[0:64, 0:1])
                    stq = small.tile([128, 512], BF16, tag="stq")
                    nc.vector.tensor_scalar_add(stq[64:128, :], ps_q[64:128, :],
                                                bq_s[64:128, 0:1])
                    nc.sync.dma_start(QTp[1][0:64, cs], stq[64:128, :])

                    ps_k = pp_qk.tile([HC, 512], F32, tag="ps_qk")
                    for ko in range(KO):
                        nc.tensor.matmul(ps_k[:], wk_s[:, ko, :], k_in[:, ko, :],
                                         start=(ko == 0), stop=(ko == KO - 1))
                    nc.vector.tensor_scalar_add(KTp[0][0:64, cs], ps_k[0:64, :],
                                                bk_s[0:64, 0:1])
                    stk = small.tile([128, 512], BF16, tag="stk")
                    nc.vector.tensor_scalar_add(stk[64:128, :], ps_k[64:128, :],
                                                bk_s[64:128, 0:1])
                    nc.sync.dma_start(KTp[1][0:64, cs], stk[64:128, :])

                    # V in natural layout (tokens on partitions); no bias --
                    # bv's contribution is folded into the host-side constant.
                    for sub in range(4):
                        tok0 = c * 512 + sub * 128
                        b, kt = tok0 // S, (tok0 % S) // 128
                        ps_v = pp_v.tile([128, 128], F32, tag="ps_v")
                        for ko in range(KO):
                            nc.tensor.matmul(ps_v[:],
                                             v_in[:, ko, bass.ts(sub, 128)],
                                             wv_s[:, ko, :],
                                             start=(ko == 0), stop=(ko == KO - 1))
                        for h in range(2):
                            nc.vector.tensor_copy(
                                V1[b][h][:, kt * 65: kt * 65 + 64],
                                ps_v[:, h * 64:(h + 1) * 64])

            # ---- phase B: attention + interleaved output projection ----
            with (
                tc.tile_pool(name="scp", bufs=2, space="PSUM") as scp,
                tc.tile_pool(name="avp", bufs=2, space="PSUM") as avp,
                tc.tile_pool(name="dnm", bufs=2, space="DRAM") as dnm,
                tc.tile_pool(name="exps", bufs=8) as exps,
            ):
                for b in range(B):
                    for qc in range(2):
                        q0 = b * S + qc * 1024
                        # both heads advance together per k-tile so the two
                        # K=64 score matmuls pair up in the PE array (row
                        # groups 0-63 / 64-127 run concurrently).
                        av = [avp.tile([65, 1024], F32, tag="av",
                                       name=f"av{b}{qc}{h}") for h in range(2)]
                        e_tiles = [[None] * 16 for _ in range(2)]
                        for kt in range(17):
                            if kt < 16:
                                sp = [scp.tile([128, 1024], F32, tag="sc",
                                               name=f"sp{b}{qc}{kt}{h}")
                                      for h in range(2)]
                                for half in range(2):
                                    for h in range(2):
                                        nc.tensor.matmul(
                                            sp[h][:, bass.ts(half, 512)],
                                            KTp[h][:, bass.ds(b * S + kt * 128, 128)],
                                            QTp[h][:, bass.ds(q0 + half * 512, 512)],
                                            start=True, stop=True)
                                for h in range(2):
                                    et = exps.tile([128, 1024], BF16, tag="et",
                                                   name=f"et{b}{qc}{kt}{h}")
                                    nc.scalar.activation(et[:], sp[h][:], AF.Exp,
                                                         scale=SCALE)
                                    e_tiles[h][kt] = et
                            if kt >= 1:
                                j = kt - 1
                                for h in range(2):
                                    for half in range(2):
                                        nc.tensor.matmul(
                                            av[h][:, bass.ts(half, 512)],
                                            V1[b][h][:, j * 65: j * 65 + 65],
                                            e_tiles[h][j][:, bass.ts(half, 512)],
                                            start=(j == 0), stop=(j == 15))
                        for h in range(2):
                            # move AV to SBUF right away to release the PSUM
                            # bank; the whole softmax normalization then runs
                            # off-PSUM and off-PE: reciprocal on DVE, then a
                            # partition broadcast via a DRAM bounce.
                            avs = small.tile([65, 1024], F32, tag="avs")
                            nc.vector.tensor_copy(avs[:], av[h][:])
                            # Reciprocal of the 1024 denominators: a (1, N)
                            # op uses a single DVE lane (~6.5us measured), so
                            # bounce through DRAM to respread them over 64
                            # partitions, invert lane-parallel, bounce back,
                            # and finally read back partition-broadcast.
                            dscr = dnm.tile([1, 1024], F32, tag="dscr")
                            nc.sync.dma_start(dscr[:], avs[64:65, :])
                            dsp = small.tile([64, 16], F32, tag="dsp")
                            nc.sync.dma_start(
                                dsp[:], dscr.rearrange("o (p j) -> (o p) j", p=64))
                            rsp = small.tile([64, 16], F32, tag="rsp")
                            nc.vector.reciprocal(rsp[:], dsp[:])
                            dscr2 = dnm.tile([1, 1024], F32, tag="dscr2")
                            nc.sync.dma_start(
                                dscr2.rearrange("o (p j) -> (o p) j", p=64), rsp[:])
                            bcs = small.tile([64, 1024], F32, tag="bcs")
                            nc.sync.dma_start(
                                bcs[:], dscr2[0:1, :].to_broadcast((64, 1024)))
                            if h == 0:
                                nc.vector.tensor_mul(attnT[0:64, bass.ds(q0, 1024)],
                                                     avs[0:64, :], bcs[:])
                            else:
                                tmp = small.tile([64, 1024], BF16, tag="tmp")
                                nc.vector.tensor_mul(tmp[:], avs[0:64, :], bcs[:])
                                # partition shift 0-63 -> 64-127 via sbuf DMA
                                nc.sync.dma_start(attnT[64:128, bass.ds(q0, 1024)],
                                                  tmp[:])

            # ---- phase C: output projection partial ----
            with (
                tc.tile_pool(name="fcp", bufs=2, space="PSUM") as fcp,
                tc.tile_pool(name="outst", bufs=3) as outst,
            ):
                for tt in range(T // 128):
                    fp = fcp.tile([128, D], F32, tag="fp")
                    for half in range(2):
                        hs = bass.ts(half, 512)
                        nc.tensor.matmul(fp[:, hs], attnT[:, bass.ts(tt, 128)],
                                         wo_s[:, hs], start=True, stop=True)
                    os_ = outst.tile([128, D], F32, tag="os")
                    nc.vector.tensor_copy(os_[:], fp[:])
                    nc.sync.dma_start(out[bass.ts(tt, 128), :], os_[:])

    if split:
        _split_multiwaits(nc)
    return nc


def _get_nc():
    global _NC
    if _NC is None:
        _NC = _build()
    return _NC


def _prep_in_maps(q, k, v, Wq, bq, Wk, bk, Wv, bv, Wo, bo):
    bf = ml_dtypes.bfloat16
    qT = np.ascontiguousarray(q.reshape(T, D).T).astype(bf)
    kT = np.ascontiguousarray(k.reshape(T, D).T).astype(bf)
    vT = np.ascontiguousarray(v.reshape(T, D).T).astype(bf)
    in_maps = []
    for c in range(NCORES):
        rows = slice(c * HC, (c + 1) * HC)
        in_maps.append({
            "qT": qT, "kT": kT, "vT": vT,
            "wq": np.ascontiguousarray(Wq[rows, :].T).astype(bf),
            "wk": np.ascontiguousarray(Wk[rows, :].T).astype(bf),
            "wv": np.ascontiguousarray(Wv[rows, :].T).astype(bf),
            "bq": np.ascontiguousarray(bq[rows]).astype(np.float32).reshape(HC, 1),
            "bk": np.ascontiguousarray(bk[rows]).astype(np.float32).reshape(HC, 1),
            "wo": np.ascontiguousarray(Wo[:, rows].T).astype(bf),
        })
    return in_maps


def _run(inputs, trace=False):
    inputs = {k_: np.asarray(v_) for k_, v_ in inputs.items()}
    nc = _get_nc()
    in_maps = _prep_in_maps(**inputs)
    res = run_bass_kernel_spmd(nc, in_maps, core_ids=list(range(NCORES)),
                               trace=trace)
    acc = np.zeros((T, D), np.float64)
    for c in range(NCORES):
        acc += res.results[c]["out"].astype(np.float64)
    const = (inputs["bo"].astype(np.float64)
             + inputs["bv"].astype(np.float64) @ inputs["Wo"].astype(np.float64).T)
    acc += const[None, :]
    return acc.reshape(B, S, D).astype(np.float32), res


def kernel(**inputs) -> np.ndarray:
    return _run(inputs)[0]

